# revision 30
# baseline (speedup 1.0000x reference)
"""GQA causal-attention prefill kernel for 8 Trainium2 NeuronCores.

Sharding: core c -> (batch b = c//4, kv head g = c%4).
Replica groups [[0,1,2,3],[4,5,6,7]] (one per batch).

Per-core pipeline (everything feature-major / "transposed" so the token dim
is always the matmul moving dim, full fp32r rate):
  1. k^T/v^T/q^T projections (+bias) from x^T, contraction over D=3584
  2. RoPE on k then q (7 heads), in-place on DVE
  3. v^T -> v (natural) via PE transposes (P@V needs keys on partitions)
  4. per head: S^T = k^T-chunk.T @ q^T (causal chunks only), +tri-mask on
     diagonal chunks, exp on ACT (no max subtraction: |logits| is small),
     denominator = ones.T @ E^T on PE, O^T_unnorm = v-chunk.T @ E^T,
     normalize via reciprocal + PE outer-product broadcast; per-head
     AllGather of the O^T block overlaps the remaining heads' compute
  5. o_proj: two m-group passes accumulating over heads in AG arrival
     order, so only the last head's gather sits near the critical path
Output per core: y[b][:, 896g:896(g+1)].T, host concatenates + transposes.
"""
import sys

if '/opt/trn_rl_repo' not in sys.path:
    sys.path.insert(0, '/opt/trn_rl_repo')

import ml_dtypes
import numpy as np

B, T, D = 2, 1024, 3584
NUM_HEADS, HEAD_DIM, NUM_KV = 28, 128, 4
REP = NUM_HEADS // NUM_KV            # 7
ROPE_THETA = 1000000.0
K_MASK = -3.3895313892515355e+38     # bf16 finfo min, as in the reference
SCALE = HEAD_DIM ** -0.5
GROUP = 4                            # tensor-parallel group size (kv heads)
NCORES = 8
DK = D // 128                        # 28 contraction chunks over D
NT = T // 512                        # token 512-tiles
SK = T // 128                        # key 128-chunks
RG = [[0, 1, 2, 3], [4, 5, 6, 7]]

_CACHE = {}


def _build_nc():
    """Build the SPMD Bass program (same program on all 8 cores)."""
    import concourse.tile as tile
    from concourse import bacc, mybir
    from concourse.masks import make_identity

    FP32 = mybir.dt.float32
    FP32R = mybir.dt.float32r
    BF16 = mybir.dt.bfloat16
    Exp = mybir.ActivationFunctionType.Exp
    Ident = mybir.ActivationFunctionType.Identity
    mult = mybir.AluOpType.mult
    addop = mybir.AluOpType.add

    nc = bacc.Bacc("TRN2", target_bir_lowering=False, debug=False, num_devices=NCORES)

    xt = nc.dram_tensor("xt", [D, T], FP32R, kind="ExternalInput")
    wq = nc.dram_tensor("wq", [D, REP * 128], FP32R, kind="ExternalInput")
    wk = nc.dram_tensor("wk", [D, 128], FP32R, kind="ExternalInput")
    wv = nc.dram_tensor("wv", [D, 128], FP32R, kind="ExternalInput")
    wo = nc.dram_tensor("wo", [D, REP * 128], BF16, kind="ExternalInput")
    bqkv = nc.dram_tensor("bqkv", [REP + 2, 128], FP32, kind="ExternalInput")
    sincat = nc.dram_tensor("sincat", [128, T], FP32, kind="ExternalInput")
    coscat = nc.dram_tensor("coscat", [128, T], FP32, kind="ExternalInput")
    trimask = nc.dram_tensor("trimask", [128, 128], FP32, kind="ExternalInput")
    onescol = nc.dram_tensor("onescol", [128, 1], FP32R, kind="ExternalInput")
    onesrow = nc.dram_tensor("onesrow", [1, 128], FP32, kind="ExternalInput")
    yt = nc.dram_tensor("yt", [REP * 128, T], FP32, kind="ExternalOutput")

    with tile.TileContext(nc) as tc:
        with (
            tc.tile_pool(name="consts", bufs=1) as consts,
            tc.tile_pool(name="qkv", bufs=1) as qkv,
            tc.tile_pool(name="dram", bufs=1, space="DRAM") as dram,
            tc.tile_pool(name="ep", bufs=4) as ep,
        ):
            tri_sb = consts.tile([128, 128], FP32, tag="tri")
            ones_col = consts.tile([128, 1], FP32R, tag="onescol")
            ones_row = consts.tile([1, 128], FP32, tag="onesrow")
            bias_sb = consts.tile([128, REP + 2], FP32, tag="bias")
            nc.sync.dma_start(tri_sb[:], trimask[:])
            nc.sync.dma_start(ones_col[:], onescol[:])
            nc.sync.dma_start(ones_row[:], onesrow[:])
            nc.sync.dma_start(bias_sb[:], bqkv.rearrange("m p -> p m"))

            q_sb = qkv.tile([128, REP, T], FP32R, tag="q")
            k_sb = qkv.tile([128, T], FP32R, tag="k")
            vn_sb = qkv.tile([128, SK, 128], FP32R, tag="vn")

            # head-group DRAM blocks for the pipelined AllGather (bf16)
            AGH = [(0, 1), (1, 3), (3, 5), (5, 7)]     # [lo, hi) head ranges
            og = [dram.tile([(hi - lo) * 128, T], BF16, tag=f"og{i}", name=f"og{i}")
                  for i, (lo, hi) in enumerate(AGH)]
            oag = [dram.tile([GROUP * (hi - lo) * 128, T], BF16,
                             tag=f"oag{i}", name=f"oag{i}")
                   for i, (lo, hi) in enumerate(AGH)]

            # warmup collective: absorbs first-op CC-stream setup cost and
            # aligns cores while P1 computes
            wu_in = dram.tile([128, 16], BF16, tag="wuin", name="wuin")
            wu_out = dram.tile([GROUP * 128, 16], BF16, tag="wuout", name="wuout")
            wu_sb = consts.tile([128, 16], BF16, tag="wusb")
            nc.vector.memset(wu_sb[:], 0.0)
            nc.sync.dma_start(wu_in[:], wu_sb[:])
            nc.gpsimd.collective_compute(
                "AllGather",
                mybir.AluOpType.bypass,
                replica_groups=RG,
                ins=[wu_in[:].opt()],
                outs=[wu_out[:].opt()],
            )

            # ---- Phase 1: projections (k, v first, then q heads) --------
            with (
                tc.tile_pool(name="xp", bufs=1) as xp,
                tc.tile_pool(name="wp", bufs=4) as wp,
                tc.tile_pool(name="vp", bufs=1) as vp,
                tc.tile_pool(name="ropep", bufs=2) as ropep,
                tc.tile_pool(name="sincosp", bufs=1) as sincosp,
                tc.tile_pool(name="pp1", bufs=2, space="PSUM") as pp1,
            ):
                sin_sb = sincosp.tile([128, T], FP32, tag="sin")
                cos_sb = sincosp.tile([128, T], FP32, tag="cos")
                nc.sync.dma_start(sin_sb[:], sincat[:])
                nc.sync.dma_start(cos_sb[:], coscat[:])
                id_sb = vp.tile([128, 128], FP32, tag="ident")
                make_identity(nc, id_sb[:])
                v_sb = vp.tile([128, T], FP32, tag="v")

                x_sb = xp.tile([128, DK, T], FP32R, tag="x")
                xr = xt.rearrange("(c p) t -> p c t", p=128)
                for cq in range(7):          # 7 DMAs of 4 chunks each
                    nc.sync.dma_start(
                        x_sb[:, 4 * cq:4 * cq + 4, :], xr[:, 4 * cq:4 * cq + 4, :]
                    )

                wqr = wq.rearrange("(c p) n -> p c n", p=128)
                wkr = wk.rearrange("(c p) n -> p c n", p=128)
                wvr = wv.rearrange("(c p) n -> p c n", p=128)

                def rope(X_full, n):
                    X = X_full[:, 512 * n:512 * (n + 1)]
                    tmp = ropep.tile([128, 512], FP32, tag="ropetmp")
                    nc.vector.tensor_copy(tmp[0:64, :], X[64:128, :])
                    nc.vector.tensor_copy(tmp[64:128, :], X[0:64, :])
                    ssl = (slice(None), slice(512 * n, 512 * (n + 1)))
                    nc.vector.tensor_tensor(tmp[:], tmp[:], sin_sb[ssl], op=mult)
                    nc.vector.tensor_tensor(X, X, cos_sb[ssl], op=mult)
                    nc.vector.tensor_tensor(X, X, tmp[:], op=addop)

                # m: 0 = k, 1 = v, 2.. = q heads 0..6
                for m in range(REP + 2):
                    wtiles = []
                    for quarter in range(4):
                        wt = wp.tile([128, 7, 128], FP32R, tag="w")
                        c0 = 7 * quarter
                        if m == 0:
                            src = wkr[:, c0:c0 + 7, :]
                        elif m == 1:
                            src = wvr[:, c0:c0 + 7, :]
                        else:
                            src = wqr[:, c0:c0 + 7, 128 * (m - 2):128 * (m - 1)]
                        nc.sync.dma_start(wt[:], src)
                        wtiles.append(wt)
                    for n in range(NT):
                        ps = pp1.tile([128, 512], FP32, tag="proj")
                        for kc in range(DK):
                            nc.tensor.matmul(
                                ps[:],
                                wtiles[kc // 7][:, kc % 7, :],
                                x_sb[:, kc, 512 * n:512 * (n + 1)],
                                start=(kc == 0),
                                stop=(kc == DK - 1),
                            )
                        if m == 0:
                            dst, bi = k_sb[:, 512 * n:512 * (n + 1)], 7
                        elif m == 1:
                            dst, bi = v_sb[:, 512 * n:512 * (n + 1)], 8
                        else:
                            dst, bi = q_sb[:, m - 2, 512 * n:512 * (n + 1)], m - 2
                        nc.scalar.activation(
                            dst, ps[:], Ident, bias=bias_sb[:, bi:bi + 1], scale=1.0
                        )
                        if m == 0:
                            rope(k_sb, n)
                        elif m == 1:
                            # v^T chunk -> v natural while v proj streams
                            for sc in range(4 * n, 4 * n + 4):
                                tp = pp1.tile([128, 128], FP32, tag="tr")
                                nc.tensor.transpose(
                                    tp[:], v_sb[:, 128 * sc:128 * (sc + 1)], id_sb[:]
                                )
                                nc.scalar.copy(vn_sb[:, sc, :], tp[:])
                        else:
                            rope(q_sb[:, m - 2, :], n)

            # ---- Phase 4: attention per head + pipelined AllGather ------
            otp_ctx = tc.tile_pool(name="otp", bufs=1)
            otp = otp_ctx.__enter__()
            otf = otp.tile([128, DK, T], BF16, tag="otf")
            ppatt_ctx = tc.tile_pool(name="ppatt", bufs=1, space="PSUM")
            ppatt = ppatt_ctx.__enter__()
            pending = []

            def finalize(h, tau, den, ops):
                rec = ep.tile([1, 512], FP32, tag="rec", name=f"rec_{h}_{tau}")
                nc.vector.reciprocal_approx_fast(rec[:], den[0:1, :])
                bc = ppatt.tile([128, 512], FP32, tag=f"den{tau % 2}",
                                name=f"bc_{h}_{tau}")
                nc.tensor.matmul(bc[:], ones_row[:], rec[:], start=True, stop=True)
                bcs = ep.tile([128, 512], FP32, tag="bcs", name=f"bcs_{h}_{tau}")
                nc.scalar.copy(bcs[:], bc[:])
                ost = ep.tile([128, 512], BF16, tag="ost", name=f"ost_{h}_{tau}")
                nc.vector.tensor_tensor(ost[:], ops[:], bcs[:], op=mult)
                grp = next(i for i, (lo, hi) in enumerate(AGH) if lo <= h < hi)
                lo, hi = AGH[grp]
                nc.sync.dma_start(
                    og[grp][128 * (h - lo):128 * (h - lo + 1),
                            512 * tau:512 * (tau + 1)],
                    ost[:],
                )
                if tau == NT - 1 and h == hi - 1:
                    nc.gpsimd.collective_compute(
                        "AllGather",
                        mybir.AluOpType.bypass,
                        replica_groups=RG,
                        ins=[og[grp][:].opt()],
                        outs=[oag[grp][:].opt()],
                    )
                    nh = hi - lo
                    for hh in range(lo, hi):
                        for gp in range(GROUP):
                            r0 = nh * 128 * gp + 128 * (hh - lo)
                            nc.gpsimd.dma_start(
                                otf[:, 7 * gp + hh, :],
                                oag[grp][r0:r0 + 128, :],
                            )

            for h in range(REP):
                for tau in range(NT):
                    n_sc = 4 * (tau + 1)
                    den = ppatt.tile([1, 512], FP32, tag=f"den{tau % 2}",
                                     name=f"den_{h}_{tau}")
                    ops = ppatt.tile([128, 512], FP32, tag=f"opv{tau % 2}",
                                     name=f"ops_{h}_{tau}")
                    etiles = {}

                    def emit_s(c):
                        delta = 128 * c - 512 * tau
                        t0 = max(delta, 0)
                        w = 512 - t0
                        sps = ppatt.tile([128, 512], FP32, tag=f"s{c % 4}",
                                         name=f"sps_{h}_{tau}_{c}")
                        tsl = slice(512 * tau + t0, 512 * (tau + 1))
                        nc.tensor.matmul(
                            sps[:, 0:w],
                            k_sb[:, 128 * c:128 * (c + 1)],
                            q_sb[:, h, tsl],
                            start=True,
                            stop=True,
                        )
                        if delta >= 0:
                            nc.vector.tensor_tensor(
                                sps[:, 0:128], sps[:, 0:128], tri_sb[:], op=addop
                            )
                        et = ep.tile([128, 512], FP32R, tag="e",
                                     name=f"et_{h}_{tau}_{c}")
                        nc.scalar.activation(et[:, 0:w], sps[:, 0:w], Exp, scale=SCALE)
                        etiles[c] = (et, t0, w)

                    def emit_acc(c):
                        et, t0, w = etiles.pop(c)
                        nc.tensor.matmul(
                            den[0:1, t0:512], ones_col[:], et[:, 0:w],
                            start=(c == 0), stop=(c == n_sc - 1),
                        )
                        nc.tensor.matmul(
                            ops[:, t0:512], vn_sb[:, c, :], et[:, 0:w],
                            start=(c == 0), stop=(c == n_sc - 1),
                        )

                    LOOKAHEAD = 2
                    for c in range(n_sc):
                        emit_s(c)
                        if c == LOOKAHEAD and pending:
                            finalize(*pending.pop(0))
                        if c >= LOOKAHEAD:
                            emit_acc(c - LOOKAHEAD)
                    for c in range(max(0, n_sc - LOOKAHEAD), n_sc):
                        emit_acc(c)
                    pending.append((h, tau, den, ops))
                    if tau == NT - 1 and any(h == hi - 1 for _, hi in AGH):
                        while pending:
                            finalize(*pending.pop(0))

            while pending:
                finalize(*pending.pop(0))
            ppatt_ctx.__exit__(None, None, None)

            # ---- Phase 6: o_proj as per-AG-wave partial sums ------------
            # Each AG wave's contribution is an independent PSUM group,
            # added into an SBUF accumulator on the (otherwise idle) DVE as
            # soon as the wave lands -- no group waits for head 6.
            with (
                tc.tile_pool(name="wp2", bufs=16) as wp2,
                tc.tile_pool(name="yaccp", bufs=1) as yaccp,
                tc.tile_pool(name="pp6", bufs=6, space="PSUM") as pp6,
            ):
                yacc = yaccp.tile([128, REP, T], FP32, tag="yacc")
                wor = wo.rearrange("(c p) n -> p c n", p=128)
                ytr = yt.rearrange("(m p) t -> p m t", p=128)
                for wi, (lo, hi) in enumerate(AGH):
                    hgs = [7 * gp + hh for hh in range(lo, hi)
                           for gp in range(GROUP)]
                    wts = {}
                    for hg in hgs:
                        wt = wp2.tile([128, REP * 128], BF16, tag="w2",
                                      name=f"w2_{wi}_{hg}")
                        nc.sync.dma_start(wt[:], wor[:, hg, :])
                        wts[hg] = wt
                    for m in range(REP):
                        for n in range(NT):
                            ps = pp6.tile([128, 512], FP32, tag="y",
                                          name=f"y_{wi}_{m}_{n}")
                            for j, hg in enumerate(hgs):
                                nc.tensor.matmul(
                                    ps[:],
                                    wts[hg][:, 128 * m:128 * (m + 1)],
                                    otf[:, hg, 512 * n:512 * (n + 1)],
                                    start=(j == 0),
                                    stop=(j == len(hgs) - 1),
                                )
                            dst = yacc[:, m, 512 * n:512 * (n + 1)]
                            if wi == 0:
                                nc.scalar.copy(dst, ps[:])
                            else:
                                nc.vector.tensor_tensor(dst, dst, ps[:], op=addop)
                        if wi == len(AGH) - 1:
                            nc.sync.dma_start(ytr[:, m, :], yacc[:, m, :])
            otp_ctx.__exit__(None, None, None)

    nc.compile()
    return nc


def _tf32_round(a):
    """Round fp32 -> tf32 (fp32r) representable values, round-to-nearest-even."""
    u = np.ascontiguousarray(a, dtype=np.float32).view(np.uint32)
    u = (u + 0xFFF + ((u >> 13) & 1)) & np.uint32(0xFFFFE000)
    return u.view(np.float32)


def _host_prep(x, segment_ids, Wq, bq, Wk, bk, Wv, bv, Wo):
    """Numpy-side input prep: transpose x, slice weights, RoPE tables, mask."""
    valid = (segment_ids != 0)
    pos = (np.cumsum(valid, axis=-1) - 1).astype(np.int32)  # CUR_IND = 0
    half = HEAD_DIM // 2
    fraction = np.arange(half, dtype=np.float32) / half
    timescale = ROPE_THETA ** fraction
    ang = pos[..., None].astype(np.float32) / timescale      # (B, T, 64)
    sin = np.sin(ang).astype(np.float32)
    cos = np.cos(ang).astype(np.float32)

    sl = np.arange(128)
    tri = np.where(sl[None, :] >= sl[:, None], 0.0, K_MASK).astype(np.float32)

    in_maps = []
    for c in range(NCORES):
        b, g = c // GROUP, c % GROUP
        qcols = slice(REP * 128 * g, REP * 128 * (g + 1))
        kvcols = slice(128 * g, 128 * (g + 1))
        bias = np.concatenate(
            [bq[qcols].reshape(REP, 128), bk[kvcols][None, :], bv[kvcols][None, :]],
            axis=0,
        ).astype(np.float32)
        sincat = np.concatenate([-sin[b].T, sin[b].T], axis=0)  # (128, T)
        coscat = np.concatenate([cos[b].T, cos[b].T], axis=0)
        in_maps.append({
            "xt": _tf32_round(np.ascontiguousarray(x[b].T, dtype=np.float32)),
            "wq": _tf32_round(np.ascontiguousarray(Wq[:, qcols], dtype=np.float32)),
            "wk": _tf32_round(np.ascontiguousarray(Wk[:, kvcols], dtype=np.float32)),
            "wv": _tf32_round(np.ascontiguousarray(Wv[:, kvcols], dtype=np.float32)),
            "wo": np.ascontiguousarray(Wo[:, qcols], dtype=np.float32).astype(ml_dtypes.bfloat16),
            "bqkv": bias,
            "sincat": np.ascontiguousarray(sincat, dtype=np.float32),
            "coscat": np.ascontiguousarray(coscat, dtype=np.float32),
            "trimask": tri,
            "onescol": np.ones((128, 1), np.float32),
            "onesrow": np.ones((1, 128), np.float32),
        })
    return in_maps


def _assemble(results):
    y = np.empty((B, T, D), dtype=np.float32)
    for b in range(B):
        blocks = [results[GROUP * b + g]["yt"] for g in range(GROUP)]
        y[b] = np.concatenate(blocks, axis=0).T
    return y


def kernel(x, segment_ids, k_cache, v_cache, Wq, bq, Wk, bk, Wv, bv, Wo,
           _trace=False, _trace_kwargs=None):
    # k_cache/v_cache are zero-initialized and fully overwritten by this
    # prefill (CUR_IND=0, cache_size==T), so they do not affect the output.
    from concourse.bass_utils import run_bass_kernel_spmd

    in_maps = _host_prep(
        np.asarray(x), np.asarray(segment_ids),
        np.asarray(Wq), np.asarray(bq), np.asarray(Wk), np.asarray(bk),
        np.asarray(Wv), np.asarray(bv), np.asarray(Wo),
    )
    if "nc" not in _CACHE:
        _CACHE["nc"] = _build_nc()
    kw = {}
    if _trace:
        kw.update(trace=True, **(_trace_kwargs or {}))
    br = run_bass_kernel_spmd(_CACHE["nc"], in_maps, core_ids=list(range(NCORES)), **kw)
    y = _assemble(br.results)
    if _trace:
        _CACHE["last_result"] = br
    return y
